# revision 16
# baseline (speedup 1.0000x reference)
"""Distributed Trainium2 (8 NeuronCores) Bass kernel for AdaptivePPOPolicyGNN.

Strategy (row-parallel dense GCN per the sharding hint):
 - Host prep: build the dense adjacency A (set semantics + self loops, values
   {0,1,2} -> exact in bf16), ship each core its row-block TRANSPOSED
   (AT_c = A[rows_c,:].T, [8192,1024] bf16) plus a replicated node-feature
   copy and tiny replicated/col-sharded MLP weights.
 - Device, per core: deg_c = row sums of A_c (PE ones-matmul while A streams
   into SBUF), AllGather deg -> dinv = rsqrt(deg); SpMM1/SpMM2 as
   feature-major TensorE matmuls out^T = xs^T @ A_c^T with xs stationary and
   A_c^T (resident in SBUF) as the moving operand; AllGather of the scaled
   hidden state between layers; exact streaming-softmax pooling combine via a
   130-float AllGather; actor logits col-sharded + AllGather; critic local.

Outputs: (action_probs [8192] f32, value scalar f32).
"""
import numpy as np
import ml_dtypes

from concourse import bass, bacc, mybir, tile, masks, bass_isa
from concourse.bass_utils import run_bass_kernel_spmd

BF16 = mybir.dt.bfloat16
FP32 = mybir.dt.float32
AF = mybir.ActivationFunctionType
ALU = mybir.AluOpType

N, F, H, A_SZ = 8192, 64, 128, 8192
C, RPC = 8, 1024          # cores, rows per core
KT, MT = N // 128, RPC // 128  # 64 k-tiles, 8 m-tiles
LN_EPS = 1e-5
GROUPS = [list(range(C))]


def build_nc(stage=7):
    nc = bacc.Bacc(None, target_bir_lowering=False, num_devices=C)

    # ---- per-core external inputs -------------------------------------
    at = nc.declare_dram_parameter("at", [N, RPC], BF16, False)        # A[rows_c].T
    xsw = nc.declare_dram_parameter("xsw", [128, KT, F], BF16, False)  # x swizzled (p,t,f)
    xtc = nc.declare_dram_parameter("xtc", [F, RPC], FP32, False)      # x[rows_c].T
    g1w = nc.declare_dram_parameter("g1w", [F, H], FP32, False)
    g1b = nc.declare_dram_parameter("g1b", [H, 1], FP32, False)
    g2w = nc.declare_dram_parameter("g2w", [H, H], FP32, False)
    g2b = nc.declare_dram_parameter("g2b", [H, 1], FP32, False)
    resw = nc.declare_dram_parameter("resw", [F, H], FP32, False)
    resb = nc.declare_dram_parameter("resb", [H, 1], FP32, False)
    a1w = nc.declare_dram_parameter("a1w", [H, F], FP32, False)
    a1b = nc.declare_dram_parameter("a1b", [F, 1], FP32, False)
    a2wc = nc.declare_dram_parameter("a2wc", [F, RPC], FP32, False)    # a2_W col shard
    a2bc = nc.declare_dram_parameter("a2bc", [1, RPC], FP32, False)
    lng = nc.declare_dram_parameter("lng", [H, 1], FP32, False)
    lnb = nc.declare_dram_parameter("lnb", [H, 1], FP32, False)
    c1w = nc.declare_dram_parameter("c1w", [H, F], FP32, False)
    c1b = nc.declare_dram_parameter("c1b", [1, F], FP32, False)
    c2w = nc.declare_dram_parameter("c2w", [1, F], FP32, False)        # c2_W.T
    c2b = nc.declare_dram_parameter("c2b", [1, 1], FP32, False)
    out_probs = nc.declare_dram_parameter("probs", [C, RPC], FP32, True)
    out_value = nc.declare_dram_parameter("value", [1, 1], FP32, True)

    # ---- collective bounce buffers ------------------------------------
    deg_in = nc.dram_tensor("deg_in", [RPC], FP32)
    deg_out = nc.dram_tensor("deg_out", [N], FP32, addr_space="Shared")
    hs_in = nc.dram_tensor("hs_in", [128, MT, H], BF16)
    hs_out = nc.dram_tensor("hs_out", [C, 128, MT, H], BF16, addr_space="Shared")
    pool_in = nc.dram_tensor("pool_in", [130], FP32)
    pool_out = nc.dram_tensor("pool_out", [C, 130], FP32, addr_space="Shared")
    lg_in = nc.dram_tensor("lg_in", [RPC], FP32)
    lg_out = nc.dram_tensor("lg_out", [C, RPC], FP32, addr_space="Shared")

    def body(tc):
        with (
            tc.tile_pool(name="per", bufs=1) as per,      # persistent sbuf
            tc.tile_pool(name="wk", bufs=4) as wkp,       # recycled f32 [*,1024] tiles
            tc.tile_pool(name="psA", bufs=3, space="PSUM") as psA,   # 2-bank psums
            tc.tile_pool(name="psB", bufs=2, space="PSUM") as psB,   # 1-bank psums
        ):
            def wk(shape, dtype=FP32, name="wkt"):
                return wkp.tile(shape, dtype, tag="wk", name=name)

            # constants
            ident = per.tile([128, 128], FP32, name="ident")
            masks.make_identity(nc, ident[:, :])
            ones_bf = per.tile([128, 1], BF16, name="ones_bf")
            nc.vector.memset(ones_bf[:, :], 1.0)
            ones_f = per.tile([128, 1], FP32, name="ones_f")
            nc.vector.memset(ones_f[:, :], 1.0)
            ones8 = per.tile([8, 1], FP32, name="ones8")
            nc.vector.memset(ones8[:, :], 1.0)
            row1 = per.tile([1, 128], FP32, name="row1")
            nc.vector.memset(row1[:, :], 1.0)

            # small weight loads
            g1w_s = per.tile([F, H], FP32, name="g1w_s"); nc.sync.dma_start(g1w_s[:, :], g1w[:, :])
            g2w_s = per.tile([H, H], FP32, name="g2w_s"); nc.sync.dma_start(g2w_s[:, :], g2w[:, :])
            resw_s = per.tile([F, H], FP32, name="resw_s"); nc.sync.dma_start(resw_s[:, :], resw[:, :])
            g1b_s = per.tile([H, 1], FP32, name="g1b_s"); nc.sync.dma_start(g1b_s[:, :], g1b[:, :])
            g2b_s = per.tile([H, 1], FP32, name="g2b_s"); nc.sync.dma_start(g2b_s[:, :], g2b[:, :])
            resb_s = per.tile([H, 1], FP32, name="resb_s"); nc.sync.dma_start(resb_s[:, :], resb[:, :])
            a1w_s = per.tile([H, F], FP32, name="a1w_s"); nc.sync.dma_start(a1w_s[:, :], a1w[:, :])
            a1b_s = per.tile([F, 1], FP32, name="a1b_s"); nc.sync.dma_start(a1b_s[:, :], a1b[:, :])
            a2wc_s = per.tile([F, RPC], FP32, name="a2wc_s"); nc.sync.dma_start(a2wc_s[:, :], a2wc[:, :])
            a2bc_s = per.tile([1, RPC], FP32, name="a2bc_s"); nc.sync.dma_start(a2bc_s[:, :], a2bc[:, :])
            lng_s = per.tile([H, 1], FP32, name="lng_s"); nc.sync.dma_start(lng_s[:, :], lng[:, :])
            lnb_s = per.tile([H, 1], FP32, name="lnb_s"); nc.sync.dma_start(lnb_s[:, :], lnb[:, :])
            c1w_s = per.tile([H, F], FP32, name="c1w_s"); nc.sync.dma_start(c1w_s[:, :], c1w[:, :])
            c1b_s = per.tile([1, F], FP32, name="c1b_s"); nc.sync.dma_start(c1b_s[:, :], c1b[:, :])
            c2w_s = per.tile([1, F], FP32, name="c2w_s"); nc.sync.dma_start(c2w_s[:, :], c2w[:, :])
            c2b_s = per.tile([1, 1], FP32, name="c2b_s"); nc.sync.dma_start(c2b_s[:, :], c2b[:, :])

            xsw_s = per.tile([128, KT, F], BF16, name="xsw_s")
            nc.sync.dma_start(xsw_s[:, :, :], xsw[:, :, :])
            xtc_s = per.tile([F, RPC], FP32, name="xtc_s")
            nc.sync.dma_start(xtc_s[:, :], xtc[:, :])

            # ---- phase 0: stream A in, fold deg = ones^T @ AT on PE ----
            at_s = per.tile([128, KT, RPC], BF16, name="at_s")
            at_r = at[:, :].rearrange("(t p) m -> p t m", p=128)
            CH = 4  # k-tiles per DMA chunk (1 MiB)
            for i in range(KT // CH):
                nc.sync.dma_start(at_s[:, i * CH:(i + 1) * CH, :], at_r[:, i * CH:(i + 1) * CH, :])
            def finish_dbg(row_ap):
                # debug early-exit: write a [1, RPC] f32 row into probs[0]
                nc.gpsimd.dma_start(out_probs[0:1, :], row_ap)
            ps_deg = psA.tile([1, RPC], FP32, tag="mm", name="ps_deg")
            for t in range(KT):
                for h in range(2):
                    nc.tensor.matmul(ps_deg[:, h * 512:(h + 1) * 512], ones_bf[:, :],
                                     at_s[:, t, h * 512:(h + 1) * 512],
                                     start=(t == 0), stop=(t == KT - 1))
            deg_row = wk([1, RPC], name="deg_row")
            nc.vector.tensor_copy(deg_row[:, :], ps_deg[:, :])
            if stage <= 1:
                finish_dbg(deg_row[:, :])
                return
            nc.gpsimd.dma_start(deg_in[:], deg_row[:, :])
            nc.gpsimd.collective_compute("AllGather", ALU.bypass, replica_groups=GROUPS,
                                         ins=[deg_in.ap().opt()], outs=[deg_out.ap().opt()])

            # ---- phase 1: dinv, D2, xs --------------------------------
            degT = per.tile([64, 128], FP32, name="degT")
            nc.gpsimd.dma_start(degT[:, :], deg_out[:].rearrange("(q j) -> q j", j=128))
            ps_t64 = psB.tile([128, 64], FP32, tag="tr", name="ps_t64")
            nc.tensor.transpose(ps_t64[:, :], degT[:, :], ident[0:64, 0:64])
            sq_t = per.tile([128, KT], FP32, name="sq_t")
            nc.scalar.activation(sq_t[:, :], ps_t64[:, :], AF.Sqrt)
            dinv_t = per.tile([128, KT], FP32, name="dinv_t")
            nc.vector.reciprocal(dinv_t[:, :], sq_t[:, :])

            dsq_row = wk([1, RPC], name="dsq_row")
            nc.scalar.activation(dsq_row[:, :], deg_row[:, :], AF.Sqrt)
            dinv_row = wk([1, RPC], name="dinv_row")
            nc.vector.reciprocal(dinv_row[:, :], dsq_row[:, :])
            ps_d2 = psA.tile([128, RPC], FP32, tag="mm", name="ps_d2")
            for h in range(2):
                nc.tensor.matmul(ps_d2[:, h * 512:(h + 1) * 512], row1[:, :],
                                 dinv_row[:, h * 512:(h + 1) * 512])
            d2_s = per.tile([128, RPC], FP32, name="d2_s")
            nc.vector.tensor_copy(d2_s[:, :], ps_d2[:, :])

            xs_s = per.tile([128, KT, F], BF16, name="xs_s")
            for t in range(KT):
                nc.vector.tensor_scalar_mul(xs_s[:, t, :], xsw_s[:, t, :], dinv_t[:, t:t + 1])

            if stage <= 2:
                drow = wk([1, RPC], name="drow")
                nc.vector.tensor_copy(drow[0:1, 0:KT], dinv_t[0:1, :])
                nc.vector.tensor_copy(drow[0:1, KT:2 * KT], d2_s[0:1, 0:KT])
                nc.vector.tensor_copy(drow[0:1, 128:1024], d2_s[0:1, 128:1024])
                finish_dbg(drow[:, :])
                return

            # ---- phase 2: SpMM1 + layer 1 -----------------------------
            ps_ax = psA.tile([F, RPC], FP32, tag="mm", name="ps_ax")
            for t in range(KT):
                for h in range(2):
                    nc.tensor.matmul(ps_ax[:, h * 512:(h + 1) * 512], xs_s[:, t, :],
                                     at_s[:, t, h * 512:(h + 1) * 512],
                                     start=(t == 0), stop=(t == KT - 1))
            axt = wk([F, RPC], name="axt")
            nc.vector.tensor_copy(axt[:, :], ps_ax[:, :])

            ps_res = psA.tile([H, RPC], FP32, tag="mm", name="ps_res")
            for h in range(2):
                nc.tensor.matmul(ps_res[:, h * 512:(h + 1) * 512], resw_s[:, :],
                                 xtc_s[:, h * 512:(h + 1) * 512])
            ps_g1 = psA.tile([H, RPC], FP32, tag="mm", name="ps_g1")
            for h in range(2):
                nc.tensor.matmul(ps_g1[:, h * 512:(h + 1) * 512], g1w_s[:, :],
                                 axt[:, h * 512:(h + 1) * 512])
            u1 = wk([H, RPC], name="u1")
            nc.vector.scalar_tensor_tensor(u1[:, :], ps_g1[:, :], 1.0, d2_s[:, :], ALU.mult, ALU.mult)
            r1 = wk([H, RPC], name="r1")
            nc.scalar.activation(r1[:, :], u1[:, :], AF.Relu, bias=g1b_s[:, 0:1])
            h1t = wk([H, RPC], name="h1t")
            nc.vector.scalar_tensor_tensor(h1t[:, :], ps_res[:, :], resb_s[:, 0:1], r1[:, :], ALU.add, ALU.add)
            hst = wk([H, RPC], name="hst")
            nc.vector.tensor_mul(hst[:, :], h1t[:, :], d2_s[:, :])

            hs_nm = per.tile([128, MT, H], BF16, name="hs_nm")
            for mt in range(MT):
                ps_tr = psB.tile([128, 128], FP32, tag="tr", name="ps_tr")
                nc.tensor.transpose(ps_tr[:, :], hst[:, mt * 128:(mt + 1) * 128], ident[:, :])
                nc.vector.tensor_copy(hs_nm[:, mt, :], ps_tr[:, :])
            if stage <= 3:
                finish_dbg(hst[0:1, :])
                return
            nc.gpsimd.dma_start(hs_in[:, :, :], hs_nm[:, :, :])
            if stage == 41:
                rb_bf = per.tile([1, RPC], BF16, name="rb_bf")
                nc.gpsimd.dma_start(rb_bf[0:1, :], hs_in[0:1, :, :].rearrange("p t f -> p (t f)"))
                rb_f = wk([1, RPC], name="rb_f")
                nc.vector.tensor_copy(rb_f[:, :], rb_bf[:, :])
                finish_dbg(rb_f[:, :])
                return
            nc.gpsimd.collective_compute("AllGather", ALU.bypass, replica_groups=GROUPS,
                                         ins=[hs_in.ap().opt()], outs=[hs_out.ap().opt()])
            if stage == 42:
                rb_bf = per.tile([1, RPC], BF16, name="rb_bf")
                nc.gpsimd.dma_start(rb_bf[0:1, :], hs_out[2, 0:1, :, :].rearrange("p t f -> p (t f)"))
                rb_f = wk([1, RPC], name="rb_f")
                nc.vector.tensor_copy(rb_f[:, :], rb_bf[:, :])
                finish_dbg(rb_f[:, :])
                return
            hs_s = per.tile([128, C, MT, H], BF16, name="hs_s")
            for r in range(C):
                nc.gpsimd.dma_start(hs_s[:, r, :, :], hs_out[r, :, :, :])

            if stage <= 4:
                hrow = wk([1, RPC], name="hrow")
                nc.vector.tensor_copy(hrow[0:1, :], hs_s[0:1, 0, :, :].rearrange("p t f -> p (t f)"))
                finish_dbg(hrow[:, :])
                return

            # ---- phase 3: SpMM2 + layer 2 + pooling -------------------
            ps_o2 = psA.tile([H, RPC], FP32, tag="mm", name="ps_o2")
            for T in range(KT):
                for h in range(2):
                    nc.tensor.matmul(ps_o2[:, h * 512:(h + 1) * 512], hs_s[:, T // MT, T % MT, :],
                                     at_s[:, T, h * 512:(h + 1) * 512],
                                     start=(T == 0), stop=(T == KT - 1))
            o2 = wk([H, RPC], name="o2")
            nc.vector.tensor_copy(o2[:, :], ps_o2[:, :])
            ps_g2 = psA.tile([H, RPC], FP32, tag="mm", name="ps_g2")
            for h in range(2):
                nc.tensor.matmul(ps_g2[:, h * 512:(h + 1) * 512], g2w_s[:, :],
                                 o2[:, h * 512:(h + 1) * 512])
            u2 = wk([H, RPC], name="u2")
            nc.vector.scalar_tensor_tensor(u2[:, :], ps_g2[:, :], 1.0, d2_s[:, :], ALU.mult, ALU.mult)
            h2t = wk([H, RPC], name="h2t")
            nc.scalar.activation(h2t[:, :], u2[:, :], AF.Relu, bias=g2b_s[:, 0:1])

            ps_s = psA.tile([1, RPC], FP32, tag="mm", name="ps_s")
            for h in range(2):
                nc.tensor.matmul(ps_s[:, h * 512:(h + 1) * 512], ones_f[:, :],
                                 h2t[:, h * 512:(h + 1) * 512])
            m_c = per.tile([1, 1], FP32, name="m_c")
            nc.vector.reduce_max(m_c[:, :], ps_s[:, :], axis=mybir.AxisListType.X)
            neg_m = per.tile([1, 1], FP32, name="neg_m")
            nc.vector.tensor_scalar_mul(neg_m[:, :], m_c[:, :], -1.0)
            w_row = wk([1, RPC], name="w_row")
            den_c = per.tile([1, 1], FP32, name="den_c")
            nc.scalar.activation(w_row[:, :], ps_s[:, :], AF.Exp, bias=neg_m[:, 0:1])
            nc.vector.reduce_sum(den_c[:, :], w_row[:, :], axis=mybir.AxisListType.X)
            ps_wb = psA.tile([128, RPC], FP32, tag="mm", name="ps_wb")
            for h in range(2):
                nc.tensor.matmul(ps_wb[:, h * 512:(h + 1) * 512], row1[:, :],
                                 w_row[:, h * 512:(h + 1) * 512])
            scr = wk([H, RPC], name="scr")
            num_c = per.tile([H, 1], FP32, name="num_c")
            nc.vector.tensor_mul(scr[:, :], h2t[:, :], ps_wb[:, :])
            nc.vector.reduce_sum(num_c[:, :], scr[:, :], axis=mybir.AxisListType.X)
            ps_trn = psB.tile([1, 128], FP32, tag="tr", name="ps_trn")
            nc.tensor.matmul(ps_trn[:, :], num_c[:, :], ident[:, :])
            pool_row = per.tile([1, 130], FP32, name="pool_row")
            nc.vector.tensor_copy(pool_row[:, 0:1], m_c[:, :])
            nc.vector.tensor_copy(pool_row[:, 1:2], den_c[:, :])
            nc.vector.tensor_copy(pool_row[:, 2:130], ps_trn[:, :])
            if stage <= 5:
                finish_dbg(h2t[0:1, :])
                return
            nc.gpsimd.dma_start(pool_in[:], pool_row[:, :])
            nc.gpsimd.collective_compute("AllGather", ALU.bypass, replica_groups=GROUPS,
                                         ins=[pool_in.ap().opt()], outs=[pool_out.ap().opt()])
            pool_s = per.tile([8, 130], FP32, name="pool_s")
            nc.gpsimd.dma_start(pool_s[:, :], pool_out[:, :])

            gmax8 = per.tile([8, 1], FP32, name="gmax8")
            nc.gpsimd.partition_all_reduce(gmax8[:, :], pool_s[:, 0:1], channels=8,
                                           reduce_op=bass_isa.ReduceOp.max)
            ngmax8 = per.tile([8, 1], FP32, name="ngmax8")
            nc.vector.tensor_scalar_mul(ngmax8[:, :], gmax8[:, :], -1.0)
            w8 = per.tile([8, 1], FP32, name="w8")
            nc.scalar.activation(w8[:, :], pool_s[:, 0:1], AF.Exp, bias=ngmax8[:, 0:1])
            scaled = per.tile([8, 129], FP32, name="scaled")
            nc.vector.tensor_scalar_mul(scaled[:, :], pool_s[:, 1:130], w8[:, 0:1])
            ps_cmb = psB.tile([128, 1], FP32, tag="tr", name="ps_cmb")
            nc.tensor.matmul(ps_cmb[:, :], scaled[:, 1:129], ones8[:, :])
            ps_den = psB.tile([1, 1], FP32, tag="tr", name="ps_den")
            nc.tensor.matmul(ps_den[:, :], scaled[:, 0:1], ones8[:, :])
            den_rec = per.tile([1, 1], FP32, name="den_rec")
            nc.vector.reciprocal(den_rec[:, :], ps_den[:, :])
            den_bc = per.tile([128, 1], FP32, name="den_bc")
            nc.gpsimd.partition_broadcast(den_bc[:, :], den_rec[:, :])
            g_pm = per.tile([128, 1], FP32, name="g_pm")
            nc.vector.tensor_scalar_mul(g_pm[:, :], ps_cmb[:, :], den_bc[:, 0:1])

            if stage <= 6:
                grow = wk([1, RPC], name="grow")
                nc.vector.memset(grow[:, :], 0.0)
                nc.vector.tensor_copy(grow[0:1, 0:8], pool_s[0:1, 0:8])
                finish_dbg(grow[:, :])
                # still run critic below (no more collectives)
            # ---- actor -----------------------------------------------
            run_actor = stage >= 7
            ps_z1 = psB.tile([F, 1], FP32, tag="tr", name="ps_z1")
            nc.tensor.matmul(ps_z1[:, :], a1w_s[:, :], g_pm[:, :])
            za = per.tile([F, 1], FP32, name="za")
            nc.scalar.activation(za[:, :], ps_z1[:, :], AF.Relu, bias=a1b_s[:, 0:1])
            ps_lg = psA.tile([1, RPC], FP32, tag="mm", name="ps_lg")
            for h in range(2):
                nc.tensor.matmul(ps_lg[:, h * 512:(h + 1) * 512], za[:, :],
                                 a2wc_s[:, h * 512:(h + 1) * 512])
            lgr = wk([1, RPC], name="lgr")
            nc.vector.tensor_add(lgr[:, :], ps_lg[:, :], a2bc_s[:, :])
            if run_actor:
                nc.gpsimd.dma_start(lg_in[:], lgr[:, :])
                nc.gpsimd.collective_compute("AllGather", ALU.bypass, replica_groups=GROUPS,
                                             ins=[lg_in.ap().opt()], outs=[lg_out.ap().opt()])
            lg_s = wk([8, RPC], name="lg_s")
            if run_actor:
                nc.gpsimd.dma_start(lg_s[:, :], lg_out[:, :])
            else:
                nc.vector.memset(lg_s[:, :], 0.0)
            lmax8 = per.tile([8, 1], FP32, name="lmax8")
            nc.vector.reduce_max(lmax8[:, :], lg_s[:, :], axis=mybir.AxisListType.X)
            glm8 = per.tile([8, 1], FP32, name="glm8")
            nc.gpsimd.partition_all_reduce(glm8[:, :], lmax8[:, :], channels=8,
                                           reduce_op=bass_isa.ReduceOp.max)
            nglm8 = per.tile([8, 1], FP32, name="nglm8")
            nc.vector.tensor_scalar_mul(nglm8[:, :], glm8[:, :], -1.0)
            e8 = wk([8, RPC], name="e8")
            esum8 = per.tile([8, 1], FP32, name="esum8")
            nc.scalar.activation(e8[:, :], lg_s[:, :], AF.Exp, bias=nglm8[:, 0:1])
            nc.vector.reduce_sum(esum8[:, :], e8[:, :], axis=mybir.AxisListType.X)
            tot8 = per.tile([8, 1], FP32, name="tot8")
            nc.gpsimd.partition_all_reduce(tot8[:, :], esum8[:, :], channels=8,
                                           reduce_op=bass_isa.ReduceOp.add)
            rec8 = per.tile([8, 1], FP32, name="rec8")
            nc.vector.reciprocal(rec8[:, :], tot8[:, :])
            probs_s = wk([8, RPC], name="probs_s")
            nc.vector.tensor_scalar_mul(probs_s[:, :], e8[:, :], rec8[:, 0:1])
            if run_actor:
                nc.gpsimd.dma_start(out_probs[:, :], probs_s[:, :])

            # ---- critic ----------------------------------------------
            mu128 = per.tile([128, 1], FP32, name="mu128")
            nc.gpsimd.partition_all_reduce(mu128[:, :], g_pm[:, :], channels=128,
                                           reduce_op=bass_isa.ReduceOp.add)
            mu = per.tile([128, 1], FP32, name="mu")
            nc.vector.tensor_scalar_mul(mu[:, :], mu128[:, :], 1.0 / H)
            tdev = per.tile([128, 1], FP32, name="tdev")
            nc.vector.tensor_sub(tdev[:, :], g_pm[:, :], mu[:, :])
            sqd = per.tile([128, 1], FP32, name="sqd")
            nc.scalar.activation(sqd[:, :], tdev[:, :], AF.Square)
            var128 = per.tile([128, 1], FP32, name="var128")
            nc.gpsimd.partition_all_reduce(var128[:, :], sqd[:, :], channels=128,
                                           reduce_op=bass_isa.ReduceOp.add)
            eps_t = per.tile([128, 1], FP32, name="eps_t")
            nc.vector.memset(eps_t[:, :], float(LN_EPS))
            sdev = per.tile([128, 1], FP32, name="sdev")
            nc.scalar.activation(sdev[:, :], var128[:, :], AF.Sqrt, bias=eps_t[:, 0:1], scale=float(1.0 / H))
            rsd = per.tile([128, 1], FP32, name="rsd")
            nc.vector.reciprocal(rsd[:, :], sdev[:, :])
            zn = per.tile([128, 1], FP32, name="zn")
            nc.vector.tensor_scalar(zn[:, :], tdev[:, :], rsd[:, 0:1], lng_s[:, 0:1], ALU.mult, ALU.mult)
            zn2 = per.tile([128, 1], FP32, name="zn2")
            nc.vector.tensor_scalar_add(zn2[:, :], zn[:, :], lnb_s[:, 0:1])
            ps_c1 = psB.tile([1, F], FP32, tag="tr", name="ps_c1")
            nc.tensor.matmul(ps_c1[:, :], zn2[:, :], c1w_s[:, :])
            cr = per.tile([1, F], FP32, name="cr")
            nc.vector.tensor_add(cr[:, :], ps_c1[:, :], c1b_s[:, :])
            cr2 = per.tile([1, F], FP32, name="cr2")
            nc.vector.tensor_relu(cr2[:, :], cr[:, :])
            scrv = per.tile([1, F], FP32, name="scrv")
            valp = per.tile([1, 1], FP32, name="valp")
            nc.vector.tensor_mul(scrv[:, :], cr2[:, :], c2w_s[:, :])
            nc.vector.reduce_sum(valp[:, :], scrv[:, :], axis=mybir.AxisListType.X)
            val2 = per.tile([1, 1], FP32, name="val2")
            nc.vector.tensor_scalar_add(val2[:, :], valp[:, :], c2b_s[:, 0:1])
            nc.gpsimd.dma_start(out_value[:, :], val2[:, :])

    with tile.TileContext(nc) as tc:
        body(tc)
    nc.compile()
    return nc


_NC_CACHE = {}


def _get_nc(stage=7):
    if stage not in _NC_CACHE:
        _NC_CACHE[stage] = build_nc(stage)
    return _NC_CACHE[stage]


def _prep_in_maps(node_features, edge_index):
    bf = ml_dtypes.bfloat16
    x = np.asarray(node_features, np.float32)
    ei = np.asarray(edge_index)

    adj = np.zeros((N, N), np.float32)
    adj[ei[0], ei[1]] = 1.0
    idx = np.arange(N)
    adj[idx, idx] += 1.0
    adj_bf = adj.astype(bf)

    x_bf = x.astype(bf)
    xsw_np = np.ascontiguousarray(x_bf.reshape(KT, 128, F).transpose(1, 0, 2))

    return adj_bf, x, xsw_np


def _run(inputs, trace=False, stage=7, **kwargs):
    nc = _get_nc(stage)
    f32 = lambda a: np.ascontiguousarray(np.asarray(a, np.float32))
    node_features = inputs["node_features"]
    edge_index = inputs["edge_index"]
    adj_bf, x, xsw_np = _prep_in_maps(node_features, edge_index)

    w = {k: f32(v) for k, v in inputs.items() if k not in ("node_features", "edge_index")}
    col = lambda a: f32(a).reshape(-1, 1)

    in_maps = []
    for c in range(C):
        r0, r1_ = c * RPC, (c + 1) * RPC
        m = {
            "at": np.ascontiguousarray(adj_bf[r0:r1_, :].T),
            "xsw": xsw_np,
            "xtc": np.ascontiguousarray(x[r0:r1_, :].T),
            "g1w": w["g1_W"], "g1b": col(w["g1_b"]),
            "g2w": w["g2_W"], "g2b": col(w["g2_b"]),
            "resw": w["res_W"], "resb": col(w["res_b"]),
            "a1w": w["a1_W"], "a1b": col(w["a1_b"]),
            "a2wc": np.ascontiguousarray(w["a2_W"][:, r0:r1_]),
            "a2bc": f32(w["a2_b"][r0:r1_]).reshape(1, RPC),
            "lng": col(w["ln_g"]), "lnb": col(w["ln_b"]),
            "c1w": w["c1_W"], "c1b": f32(w["c1_b"]).reshape(1, F),
            "c2w": np.ascontiguousarray(w["c2_W"].T), "c2b": f32(w["c2_b"]).reshape(1, 1),
        }
        in_maps.append(m)

    res = run_bass_kernel_spmd(nc, in_maps, core_ids=list(range(C)), trace=trace, **kwargs)
    return res


def kernel(**inputs):
    res = _run(inputs, trace=False)
    out = res.results[0]
    probs = np.asarray(out["probs"], np.float32).reshape(A_SZ)
    value = np.float32(np.asarray(out["value"]).reshape(-1)[0])
    return probs, value


# revision 18
# speedup vs baseline: 1.0440x; 1.0440x over previous
"""Distributed Trainium2 (8 NeuronCores) Bass kernel for AdaptivePPOPolicyGNN.

Strategy (row-parallel dense GCN per the sharding hint):
 - Host prep: build the dense adjacency A (set semantics + self loops, values
   {0,1,2} -> exact in bf16), ship each core its row-block TRANSPOSED
   (AT_c = A[rows_c,:].T, [8192,1024] bf16) plus a replicated node-feature
   copy and tiny replicated/col-sharded MLP weights.
 - Device, per core: deg_c = row sums of A_c (PE ones-matmul while A streams
   into SBUF), AllGather deg -> dinv = rsqrt(deg); SpMM1/SpMM2 as
   feature-major TensorE matmuls out^T = xs^T @ A_c^T with xs stationary and
   A_c^T (resident in SBUF) as the moving operand; AllGather of the scaled
   hidden state between layers; exact streaming-softmax pooling combine via a
   130-float AllGather; actor logits col-sharded + AllGather; critic local.

Outputs: (action_probs [8192] f32, value scalar f32).
"""
import numpy as np
import ml_dtypes

from concourse import bass, bacc, mybir, tile, masks, bass_isa
from concourse.bass_utils import run_bass_kernel_spmd

BF16 = mybir.dt.bfloat16
FP32 = mybir.dt.float32
AF = mybir.ActivationFunctionType
ALU = mybir.AluOpType

N, F, H, A_SZ = 8192, 64, 128, 8192
C, RPC = 8, 1024          # cores, rows per core
KT, MT = N // 128, RPC // 128  # 64 k-tiles, 8 m-tiles
LN_EPS = 1e-5
GROUPS = [list(range(C))]


def build_nc(stage=7):
    nc = bacc.Bacc(None, target_bir_lowering=False, num_devices=C)

    # ---- per-core external inputs -------------------------------------
    at = nc.declare_dram_parameter("at", [N, RPC], BF16, False)        # A[rows_c].T
    xsw = nc.declare_dram_parameter("xsw", [128, KT, F], BF16, False)  # x swizzled (p,t,f)
    xtc = nc.declare_dram_parameter("xtc", [F, RPC], BF16, False)      # x[rows_c].T
    g1w = nc.declare_dram_parameter("g1w", [F, H], BF16, False)
    g1b = nc.declare_dram_parameter("g1b", [H, 1], FP32, False)
    g2w = nc.declare_dram_parameter("g2w", [H, H], BF16, False)
    g2b = nc.declare_dram_parameter("g2b", [H, 1], FP32, False)
    resw = nc.declare_dram_parameter("resw", [F, H], BF16, False)
    resb = nc.declare_dram_parameter("resb", [H, 1], FP32, False)
    a1w = nc.declare_dram_parameter("a1w", [H, F], FP32, False)
    a1b = nc.declare_dram_parameter("a1b", [F, 1], FP32, False)
    a2wc = nc.declare_dram_parameter("a2wc", [F, RPC], BF16, False)    # a2_W col shard
    a2bc = nc.declare_dram_parameter("a2bc", [1, RPC], FP32, False)
    lng = nc.declare_dram_parameter("lng", [H, 1], FP32, False)
    lnb = nc.declare_dram_parameter("lnb", [H, 1], FP32, False)
    c1w = nc.declare_dram_parameter("c1w", [H, F], FP32, False)
    c1b = nc.declare_dram_parameter("c1b", [1, F], FP32, False)
    c2w = nc.declare_dram_parameter("c2w", [1, F], FP32, False)        # c2_W.T
    c2b = nc.declare_dram_parameter("c2b", [1, 1], FP32, False)
    out_probs = nc.declare_dram_parameter("probs", [C, RPC], FP32, True)
    out_value = nc.declare_dram_parameter("value", [1, 1], FP32, True)

    # ---- collective bounce buffers ------------------------------------
    deg_in = nc.dram_tensor("deg_in", [RPC], FP32)
    deg_out = nc.dram_tensor("deg_out", [N], FP32, addr_space="Shared")
    hs_in = nc.dram_tensor("hs_in", [128, MT, H], BF16)
    hs_out = nc.dram_tensor("hs_out", [C, 128, MT, H], BF16, addr_space="Shared")
    pool_in = nc.dram_tensor("pool_in", [130], FP32)
    pool_out = nc.dram_tensor("pool_out", [C, 130], FP32, addr_space="Shared")
    lg_in = nc.dram_tensor("lg_in", [RPC], FP32)
    lg_out = nc.dram_tensor("lg_out", [C, RPC], FP32, addr_space="Shared")

    def body(tc):
        with (
            tc.tile_pool(name="per", bufs=1) as per,      # persistent sbuf
            tc.tile_pool(name="wk", bufs=4) as wkp,       # recycled f32 [*,1024] tiles
            tc.tile_pool(name="psA", bufs=3, space="PSUM") as psA,   # 2-bank psums
            tc.tile_pool(name="psB", bufs=2, space="PSUM") as psB,   # 1-bank psums
        ):
            def wk(shape, dtype=FP32, name="wkt"):
                return wkp.tile(shape, dtype, tag="wk", name=name)

            # constants
            ident = per.tile([128, 128], FP32, name="ident")
            masks.make_identity(nc, ident[:, :])
            ones_bf = per.tile([128, 1], BF16, name="ones_bf")
            nc.vector.memset(ones_bf[:, :], 1.0)
            ones_f = per.tile([128, 1], FP32, name="ones_f")
            nc.vector.memset(ones_f[:, :], 1.0)
            ones8 = per.tile([8, 1], FP32, name="ones8")
            nc.vector.memset(ones8[:, :], 1.0)
            row1 = per.tile([1, 128], FP32, name="row1")
            nc.vector.memset(row1[:, :], 1.0)
            row1_bf = per.tile([1, 128], BF16, name="row1_bf")
            nc.vector.memset(row1_bf[:, :], 1.0)

            # small weight loads
            g1w_s = per.tile([F, H], BF16, name="g1w_s"); nc.sync.dma_start(g1w_s[:, :], g1w[:, :])
            g2w_s = per.tile([H, H], BF16, name="g2w_s"); nc.sync.dma_start(g2w_s[:, :], g2w[:, :])
            resw_s = per.tile([F, H], BF16, name="resw_s"); nc.sync.dma_start(resw_s[:, :], resw[:, :])
            g1b_s = per.tile([H, 1], FP32, name="g1b_s"); nc.sync.dma_start(g1b_s[:, :], g1b[:, :])
            g2b_s = per.tile([H, 1], FP32, name="g2b_s"); nc.sync.dma_start(g2b_s[:, :], g2b[:, :])
            resb_s = per.tile([H, 1], FP32, name="resb_s"); nc.sync.dma_start(resb_s[:, :], resb[:, :])
            a1w_s = per.tile([H, F], FP32, name="a1w_s"); nc.sync.dma_start(a1w_s[:, :], a1w[:, :])
            a1b_s = per.tile([F, 1], FP32, name="a1b_s"); nc.sync.dma_start(a1b_s[:, :], a1b[:, :])
            a2wc_s = per.tile([F, RPC], BF16, name="a2wc_s"); nc.sync.dma_start(a2wc_s[:, :], a2wc[:, :])
            a2bc_s = per.tile([1, RPC], FP32, name="a2bc_s"); nc.sync.dma_start(a2bc_s[:, :], a2bc[:, :])
            lng_s = per.tile([H, 1], FP32, name="lng_s"); nc.sync.dma_start(lng_s[:, :], lng[:, :])
            lnb_s = per.tile([H, 1], FP32, name="lnb_s"); nc.sync.dma_start(lnb_s[:, :], lnb[:, :])
            c1w_s = per.tile([H, F], FP32, name="c1w_s"); nc.sync.dma_start(c1w_s[:, :], c1w[:, :])
            c1b_s = per.tile([1, F], FP32, name="c1b_s"); nc.sync.dma_start(c1b_s[:, :], c1b[:, :])
            c2w_s = per.tile([1, F], FP32, name="c2w_s"); nc.sync.dma_start(c2w_s[:, :], c2w[:, :])
            c2b_s = per.tile([1, 1], FP32, name="c2b_s"); nc.sync.dma_start(c2b_s[:, :], c2b[:, :])

            xsw_s = per.tile([128, KT, F], BF16, name="xsw_s")
            nc.sync.dma_start(xsw_s[:, :, :], xsw[:, :, :])
            xtc_s = per.tile([F, RPC], BF16, name="xtc_s")
            nc.sync.dma_start(xtc_s[:, :], xtc[:, :])

            # ---- phase 0: stream A in, fold deg = ones^T @ AT on PE ----
            at_s = per.tile([128, KT, RPC], BF16, name="at_s")
            at_r = at[:, :].rearrange("(t p) m -> p t m", p=128)
            CH = 8  # k-tiles per DMA chunk (2 MiB), alternate the two HWDGE rings
            for i in range(KT // CH):
                eng = nc.sync if i % 2 == 0 else nc.scalar
                eng.dma_start(at_s[:, i * CH:(i + 1) * CH, :], at_r[:, i * CH:(i + 1) * CH, :])
            def finish_dbg(row_ap):
                # debug early-exit: write a [1, RPC] f32 row into probs[0]
                nc.gpsimd.dma_start(out_probs[0:1, :], row_ap)
            # deg: even k-tiles summed on PE (ones-matmul), odd tiles on DVE (bf16
            # adds are exact for these small-integer values), both folded into ps_deg.
            ps_deg = psA.tile([1, RPC], FP32, tag="mm", name="ps_deg")
            acc = per.tile([128, RPC], BF16, name="acc")
            for t in range(KT):
                if t % 2 == 0:
                    for h in range(2):
                        nc.tensor.matmul(ps_deg[:, h * 512:(h + 1) * 512], ones_bf[:, :],
                                         at_s[:, t, h * 512:(h + 1) * 512],
                                         start=(t == 0), stop=False)
                elif t == 1:
                    nc.vector.tensor_copy(acc[:, :], at_s[:, t, :])
                else:
                    nc.vector.tensor_add(acc[:, :], acc[:, :], at_s[:, t, :])
            for h in range(2):
                nc.tensor.matmul(ps_deg[:, h * 512:(h + 1) * 512], ones_bf[:, :],
                                 acc[:, h * 512:(h + 1) * 512], start=False, stop=True)
            deg_row = wk([1, RPC], name="deg_row")
            nc.vector.tensor_copy(deg_row[:, :], ps_deg[:, :])
            if stage <= 1:
                finish_dbg(deg_row[:, :])
                return
            nc.gpsimd.dma_start(deg_in[:], deg_row[:, :])
            nc.gpsimd.collective_compute("AllGather", ALU.bypass, replica_groups=GROUPS,
                                         ins=[deg_in.ap().opt()], outs=[deg_out.ap().opt()])

            # ---- phase 1: dinv, D2, xs --------------------------------
            degT = per.tile([64, 128], FP32, name="degT")
            nc.gpsimd.dma_start(degT[:, :], deg_out[:].rearrange("(q j) -> q j", j=128))
            ps_t64 = psB.tile([128, 64], FP32, tag="tr", name="ps_t64")
            nc.tensor.transpose(ps_t64[:, :], degT[:, :], ident[0:64, 0:64])
            sq_t = per.tile([128, KT], FP32, name="sq_t")
            nc.scalar.activation(sq_t[:, :], ps_t64[:, :], AF.Ln)
            dinv_t = per.tile([128, KT], FP32, name="dinv_t")
            nc.scalar.activation(dinv_t[:, :], sq_t[:, :], AF.Exp, scale=-0.5)

            dsq_row = wk([1, RPC], name="dsq_row")
            nc.scalar.activation(dsq_row[:, :], deg_row[:, :], AF.Ln)
            dinv_row = wk([1, RPC], name="dinv_row")
            nc.scalar.activation(dinv_row[:, :], dsq_row[:, :], AF.Exp, scale=-0.5)
            ps_d2 = psA.tile([128, RPC], FP32, tag="mm", name="ps_d2")
            for h in range(2):
                nc.tensor.matmul(ps_d2[:, h * 512:(h + 1) * 512], row1[:, :],
                                 dinv_row[:, h * 512:(h + 1) * 512])
            d2_s = per.tile([128, RPC], FP32, name="d2_s")
            nc.vector.tensor_copy(d2_s[:, :], ps_d2[:, :])

            xs_s = per.tile([128, KT, F], BF16, name="xs_s")
            for t in range(KT):
                nc.vector.tensor_scalar_mul(xs_s[:, t, :], xsw_s[:, t, :], dinv_t[:, t:t + 1])

            if stage <= 2:
                drow = wk([1, RPC], name="drow")
                nc.vector.tensor_copy(drow[0:1, 0:KT], dinv_t[0:1, :])
                nc.vector.tensor_copy(drow[0:1, KT:2 * KT], d2_s[0:1, 0:KT])
                nc.vector.tensor_copy(drow[0:1, 128:1024], d2_s[0:1, 128:1024])
                finish_dbg(drow[:, :])
                return

            # ---- phase 2: SpMM1 + layer 1 -----------------------------
            ps_ax = psA.tile([F, RPC], FP32, tag="mm", name="ps_ax")
            for t in range(KT):
                for h in range(2):
                    nc.tensor.matmul(ps_ax[:, h * 512:(h + 1) * 512], xs_s[:, t, :],
                                     at_s[:, t, h * 512:(h + 1) * 512],
                                     start=(t == 0), stop=(t == KT - 1))
            axt = wk([F, RPC], BF16, name="axt")
            nc.vector.tensor_copy(axt[:, :], ps_ax[:, :])

            ps_res = psA.tile([H, RPC], FP32, tag="mm", name="ps_res")
            for h in range(2):
                nc.tensor.matmul(ps_res[:, h * 512:(h + 1) * 512], resw_s[:, :],
                                 xtc_s[:, h * 512:(h + 1) * 512])
            ps_g1 = psA.tile([H, RPC], FP32, tag="mm", name="ps_g1")
            for h in range(2):
                nc.tensor.matmul(ps_g1[:, h * 512:(h + 1) * 512], g1w_s[:, :],
                                 axt[:, h * 512:(h + 1) * 512])
            u1 = wk([H, RPC], name="u1")
            nc.vector.scalar_tensor_tensor(u1[:, :], ps_g1[:, :], 1.0, d2_s[:, :], ALU.mult, ALU.mult)
            r1 = wk([H, RPC], name="r1")
            nc.vector.tensor_scalar(r1[:, :], u1[:, :], g1b_s[:, 0:1], 0.0, ALU.add, ALU.max)
            h1t = wk([H, RPC], name="h1t")
            nc.vector.scalar_tensor_tensor(h1t[:, :], ps_res[:, :], resb_s[:, 0:1], r1[:, :], ALU.add, ALU.add)
            hst = wk([H, RPC], name="hst")
            nc.vector.tensor_mul(hst[:, :], h1t[:, :], d2_s[:, :])

            hs_nm = per.tile([128, MT, H], BF16, name="hs_nm")
            for mt in range(MT):
                ps_tr = psB.tile([128, 128], FP32, tag="tr", name="ps_tr")
                nc.tensor.transpose(ps_tr[:, :], hst[:, mt * 128:(mt + 1) * 128], ident[:, :])
                nc.vector.tensor_copy(hs_nm[:, mt, :], ps_tr[:, :])
            if stage <= 3:
                finish_dbg(hst[0:1, :])
                return
            nc.gpsimd.dma_start(hs_in[:, :, :], hs_nm[:, :, :])
            if stage == 41:
                rb_bf = per.tile([1, RPC], BF16, name="rb_bf")
                nc.gpsimd.dma_start(rb_bf[0:1, :], hs_in[0:1, :, :].rearrange("p t f -> p (t f)"))
                rb_f = wk([1, RPC], name="rb_f")
                nc.vector.tensor_copy(rb_f[:, :], rb_bf[:, :])
                finish_dbg(rb_f[:, :])
                return
            nc.gpsimd.collective_compute("AllGather", ALU.bypass, replica_groups=GROUPS,
                                         ins=[hs_in.ap().opt()], outs=[hs_out.ap().opt()])
            if stage == 42:
                rb_bf = per.tile([1, RPC], BF16, name="rb_bf")
                nc.gpsimd.dma_start(rb_bf[0:1, :], hs_out[2, 0:1, :, :].rearrange("p t f -> p (t f)"))
                rb_f = wk([1, RPC], name="rb_f")
                nc.vector.tensor_copy(rb_f[:, :], rb_bf[:, :])
                finish_dbg(rb_f[:, :])
                return
            hs_s = per.tile([128, C, MT, H], BF16, name="hs_s")
            for r in range(C):
                nc.gpsimd.dma_start(hs_s[:, r, :, :], hs_out[r, :, :, :])

            if stage <= 4:
                hrow = wk([1, RPC], name="hrow")
                nc.vector.tensor_copy(hrow[0:1, :], hs_s[0:1, 0, :, :].rearrange("p t f -> p (t f)"))
                finish_dbg(hrow[:, :])
                return

            # ---- phase 3: SpMM2 + layer 2 + pooling -------------------
            ps_o2 = psA.tile([H, RPC], FP32, tag="mm", name="ps_o2")
            for T in range(KT):
                for h in range(2):
                    nc.tensor.matmul(ps_o2[:, h * 512:(h + 1) * 512], hs_s[:, T // MT, T % MT, :],
                                     at_s[:, T, h * 512:(h + 1) * 512],
                                     start=(T == 0), stop=(T == KT - 1))
            o2 = wk([H, RPC], BF16, name="o2")
            nc.vector.tensor_copy(o2[:, :], ps_o2[:, :])
            ps_g2 = psA.tile([H, RPC], FP32, tag="mm", name="ps_g2")
            for h in range(2):
                nc.tensor.matmul(ps_g2[:, h * 512:(h + 1) * 512], g2w_s[:, :],
                                 o2[:, h * 512:(h + 1) * 512])
            u2 = wk([H, RPC], name="u2")
            nc.vector.scalar_tensor_tensor(u2[:, :], ps_g2[:, :], 1.0, d2_s[:, :], ALU.mult, ALU.mult)
            h2t = wk([H, RPC], BF16, name="h2t")
            nc.vector.tensor_scalar(h2t[:, :], u2[:, :], g2b_s[:, 0:1], 0.0, ALU.add, ALU.max)

            ps_s = psA.tile([1, RPC], FP32, tag="mm", name="ps_s")
            for h in range(2):
                nc.tensor.matmul(ps_s[:, h * 512:(h + 1) * 512], ones_bf[:, :],
                                 h2t[:, h * 512:(h + 1) * 512])
            m_c = per.tile([1, 1], FP32, name="m_c")
            nc.vector.reduce_max(m_c[:, :], ps_s[:, :], axis=mybir.AxisListType.X)
            neg_m = per.tile([1, 1], FP32, name="neg_m")
            nc.vector.tensor_scalar_mul(neg_m[:, :], m_c[:, :], -1.0)
            w_row = wk([1, RPC], BF16, name="w_row")
            den_c = per.tile([1, 1], FP32, name="den_c")
            nc.scalar.activation(w_row[:, :], ps_s[:, :], AF.Exp, bias=neg_m[:, 0:1])
            nc.vector.reduce_sum(den_c[:, :], w_row[:, :], axis=mybir.AxisListType.X)
            ps_wb = psA.tile([128, RPC], FP32, tag="mm", name="ps_wb")
            for h in range(2):
                nc.tensor.matmul(ps_wb[:, h * 512:(h + 1) * 512], row1_bf[:, :],
                                 w_row[:, h * 512:(h + 1) * 512])
            scr = wk([H, RPC], name="scr")
            num_c = per.tile([H, 1], FP32, name="num_c")
            nc.vector.tensor_mul(scr[:, :], h2t[:, :], ps_wb[:, :])
            nc.vector.reduce_sum(num_c[:, :], scr[:, :], axis=mybir.AxisListType.X)
            ps_trn = psB.tile([1, 128], FP32, tag="tr", name="ps_trn")
            nc.tensor.matmul(ps_trn[:, :], num_c[:, :], ident[:, :])
            pool_row = per.tile([1, 130], FP32, name="pool_row")
            nc.vector.tensor_copy(pool_row[:, 0:1], m_c[:, :])
            nc.vector.tensor_copy(pool_row[:, 1:2], den_c[:, :])
            nc.vector.tensor_copy(pool_row[:, 2:130], ps_trn[:, :])
            if stage <= 5:
                finish_dbg(h2t[0:1, :])
                return
            nc.gpsimd.dma_start(pool_in[:], pool_row[:, :])
            nc.gpsimd.collective_compute("AllGather", ALU.bypass, replica_groups=GROUPS,
                                         ins=[pool_in.ap().opt()], outs=[pool_out.ap().opt()])
            pool_s = per.tile([8, 130], FP32, name="pool_s")
            nc.gpsimd.dma_start(pool_s[:, :], pool_out[:, :])

            gmax8 = per.tile([8, 1], FP32, name="gmax8")
            nc.gpsimd.partition_all_reduce(gmax8[:, :], pool_s[:, 0:1], channels=8,
                                           reduce_op=bass_isa.ReduceOp.max)
            ngmax8 = per.tile([8, 1], FP32, name="ngmax8")
            nc.vector.tensor_scalar_mul(ngmax8[:, :], gmax8[:, :], -1.0)
            w8 = per.tile([8, 1], FP32, name="w8")
            nc.scalar.activation(w8[:, :], pool_s[:, 0:1], AF.Exp, bias=ngmax8[:, 0:1])
            scaled = per.tile([8, 129], FP32, name="scaled")
            nc.vector.tensor_scalar_mul(scaled[:, :], pool_s[:, 1:130], w8[:, 0:1])
            ps_cmb = psB.tile([128, 1], FP32, tag="tr", name="ps_cmb")
            nc.tensor.matmul(ps_cmb[:, :], scaled[:, 1:129], ones8[:, :])
            ps_den = psB.tile([1, 1], FP32, tag="tr", name="ps_den")
            nc.tensor.matmul(ps_den[:, :], scaled[:, 0:1], ones8[:, :])
            den_rec = per.tile([1, 1], FP32, name="den_rec")
            nc.vector.reciprocal(den_rec[:, :], ps_den[:, :])
            den_bc = per.tile([128, 1], FP32, name="den_bc")
            nc.gpsimd.partition_broadcast(den_bc[:, :], den_rec[:, :])
            g_pm = per.tile([128, 1], FP32, name="g_pm")
            nc.vector.tensor_scalar_mul(g_pm[:, :], ps_cmb[:, :], den_bc[:, 0:1])

            if stage <= 6:
                grow = wk([1, RPC], name="grow")
                nc.vector.memset(grow[:, :], 0.0)
                nc.vector.tensor_copy(grow[0:1, 0:8], pool_s[0:1, 0:8])
                finish_dbg(grow[:, :])
                # still run critic below (no more collectives)
            # ---- actor -----------------------------------------------
            run_actor = stage >= 7
            ps_z1 = psB.tile([F, 1], FP32, tag="tr", name="ps_z1")
            nc.tensor.matmul(ps_z1[:, :], a1w_s[:, :], g_pm[:, :])
            za = per.tile([F, 1], BF16, name="za")
            nc.vector.tensor_scalar(za[:, :], ps_z1[:, :], a1b_s[:, 0:1], 0.0, ALU.add, ALU.max)
            ps_lg = psA.tile([1, RPC], FP32, tag="mm", name="ps_lg")
            for h in range(2):
                nc.tensor.matmul(ps_lg[:, h * 512:(h + 1) * 512], za[:, :],
                                 a2wc_s[:, h * 512:(h + 1) * 512])
            lgr = wk([1, RPC], name="lgr")
            nc.vector.tensor_add(lgr[:, :], ps_lg[:, :], a2bc_s[:, :])
            if run_actor:
                nc.gpsimd.dma_start(lg_in[:], lgr[:, :])
                nc.gpsimd.collective_compute("AllGather", ALU.bypass, replica_groups=GROUPS,
                                             ins=[lg_in.ap().opt()], outs=[lg_out.ap().opt()])
            lg_s = wk([8, RPC], name="lg_s")
            if run_actor:
                nc.gpsimd.dma_start(lg_s[:, :], lg_out[:, :])
            else:
                nc.vector.memset(lg_s[:, :], 0.0)
            lmax8 = per.tile([8, 1], FP32, name="lmax8")
            nc.vector.reduce_max(lmax8[:, :], lg_s[:, :], axis=mybir.AxisListType.X)
            glm8 = per.tile([8, 1], FP32, name="glm8")
            nc.gpsimd.partition_all_reduce(glm8[:, :], lmax8[:, :], channels=8,
                                           reduce_op=bass_isa.ReduceOp.max)
            nglm8 = per.tile([8, 1], FP32, name="nglm8")
            nc.vector.tensor_scalar_mul(nglm8[:, :], glm8[:, :], -1.0)
            e8 = wk([8, RPC], name="e8")
            esum8 = per.tile([8, 1], FP32, name="esum8")
            nc.scalar.activation(e8[:, :], lg_s[:, :], AF.Exp, bias=nglm8[:, 0:1])
            nc.vector.reduce_sum(esum8[:, :], e8[:, :], axis=mybir.AxisListType.X)
            tot8 = per.tile([8, 1], FP32, name="tot8")
            nc.gpsimd.partition_all_reduce(tot8[:, :], esum8[:, :], channels=8,
                                           reduce_op=bass_isa.ReduceOp.add)
            rec8 = per.tile([8, 1], FP32, name="rec8")
            nc.vector.reciprocal(rec8[:, :], tot8[:, :])
            probs_s = wk([8, RPC], name="probs_s")
            nc.vector.tensor_scalar_mul(probs_s[:, :], e8[:, :], rec8[:, 0:1])
            if run_actor:
                nc.gpsimd.dma_start(out_probs[:, :], probs_s[:, :])

            # ---- critic ----------------------------------------------
            mu128 = per.tile([128, 1], FP32, name="mu128")
            nc.gpsimd.partition_all_reduce(mu128[:, :], g_pm[:, :], channels=128,
                                           reduce_op=bass_isa.ReduceOp.add)
            mu = per.tile([128, 1], FP32, name="mu")
            nc.vector.tensor_scalar_mul(mu[:, :], mu128[:, :], 1.0 / H)
            tdev = per.tile([128, 1], FP32, name="tdev")
            nc.vector.tensor_sub(tdev[:, :], g_pm[:, :], mu[:, :])
            sqd = per.tile([128, 1], FP32, name="sqd")
            nc.vector.tensor_mul(sqd[:, :], tdev[:, :], tdev[:, :])
            var128 = per.tile([128, 1], FP32, name="var128")
            nc.gpsimd.partition_all_reduce(var128[:, :], sqd[:, :], channels=128,
                                           reduce_op=bass_isa.ReduceOp.add)
            eps_t = per.tile([128, 1], FP32, name="eps_t")
            nc.vector.memset(eps_t[:, :], float(LN_EPS))
            sdev = per.tile([128, 1], FP32, name="sdev")
            nc.scalar.activation(sdev[:, :], var128[:, :], AF.Sqrt, bias=eps_t[:, 0:1], scale=float(1.0 / H))
            rsd = per.tile([128, 1], FP32, name="rsd")
            nc.vector.reciprocal(rsd[:, :], sdev[:, :])
            zn = per.tile([128, 1], FP32, name="zn")
            nc.vector.tensor_scalar(zn[:, :], tdev[:, :], rsd[:, 0:1], lng_s[:, 0:1], ALU.mult, ALU.mult)
            zn2 = per.tile([128, 1], FP32, name="zn2")
            nc.vector.tensor_scalar_add(zn2[:, :], zn[:, :], lnb_s[:, 0:1])
            ps_c1 = psB.tile([1, F], FP32, tag="tr", name="ps_c1")
            nc.tensor.matmul(ps_c1[:, :], zn2[:, :], c1w_s[:, :])
            cr = per.tile([1, F], FP32, name="cr")
            nc.vector.tensor_add(cr[:, :], ps_c1[:, :], c1b_s[:, :])
            cr2 = per.tile([1, F], FP32, name="cr2")
            nc.vector.tensor_relu(cr2[:, :], cr[:, :])
            scrv = per.tile([1, F], FP32, name="scrv")
            valp = per.tile([1, 1], FP32, name="valp")
            nc.vector.tensor_mul(scrv[:, :], cr2[:, :], c2w_s[:, :])
            nc.vector.reduce_sum(valp[:, :], scrv[:, :], axis=mybir.AxisListType.X)
            val2 = per.tile([1, 1], FP32, name="val2")
            nc.vector.tensor_scalar_add(val2[:, :], valp[:, :], c2b_s[:, 0:1])
            nc.gpsimd.dma_start(out_value[:, :], val2[:, :])

    with tile.TileContext(nc) as tc:
        body(tc)
    nc.compile()
    return nc


_NC_CACHE = {}


def _get_nc(stage=7):
    if stage not in _NC_CACHE:
        _NC_CACHE[stage] = build_nc(stage)
    return _NC_CACHE[stage]


def _prep_in_maps(node_features, edge_index):
    bf = ml_dtypes.bfloat16
    x = np.asarray(node_features, np.float32)
    ei = np.asarray(edge_index)

    adj = np.zeros((N, N), np.float32)
    adj[ei[0], ei[1]] = 1.0
    idx = np.arange(N)
    adj[idx, idx] += 1.0
    adj_bf = adj.astype(bf)

    x_bf = x.astype(bf)
    xsw_np = np.ascontiguousarray(x_bf.reshape(KT, 128, F).transpose(1, 0, 2))

    return adj_bf, x, xsw_np


def _run(inputs, trace=False, stage=7, **kwargs):
    nc = _get_nc(stage)
    f32 = lambda a: np.ascontiguousarray(np.asarray(a, np.float32))
    node_features = inputs["node_features"]
    edge_index = inputs["edge_index"]
    adj_bf, x, xsw_np = _prep_in_maps(node_features, edge_index)

    w = {k: f32(v) for k, v in inputs.items() if k not in ("node_features", "edge_index")}
    col = lambda a: f32(a).reshape(-1, 1)
    bfc = lambda a: np.ascontiguousarray(np.asarray(a, np.float32).astype(ml_dtypes.bfloat16))

    in_maps = []
    for c in range(C):
        r0, r1_ = c * RPC, (c + 1) * RPC
        m = {
            "at": np.ascontiguousarray(adj_bf[r0:r1_, :].T),
            "xsw": xsw_np,
            "xtc": bfc(x[r0:r1_, :].T),
            "g1w": bfc(w["g1_W"]), "g1b": col(w["g1_b"]),
            "g2w": bfc(w["g2_W"]), "g2b": col(w["g2_b"]),
            "resw": bfc(w["res_W"]), "resb": col(w["res_b"]),
            "a1w": w["a1_W"], "a1b": col(w["a1_b"]),
            "a2wc": bfc(w["a2_W"][:, r0:r1_]),
            "a2bc": f32(w["a2_b"][r0:r1_]).reshape(1, RPC),
            "lng": col(w["ln_g"]), "lnb": col(w["ln_b"]),
            "c1w": w["c1_W"], "c1b": f32(w["c1_b"]).reshape(1, F),
            "c2w": np.ascontiguousarray(w["c2_W"].T), "c2b": f32(w["c2_b"]).reshape(1, 1),
        }
        in_maps.append(m)

    res = run_bass_kernel_spmd(nc, in_maps, core_ids=list(range(C)), trace=trace, **kwargs)
    return res


def kernel(**inputs):
    res = _run(inputs, trace=False)
    out = res.results[0]
    probs = np.asarray(out["probs"], np.float32).reshape(A_SZ)
    value = np.float32(np.asarray(out["value"]).reshape(-1)[0])
    return probs, value


# revision 20
# speedup vs baseline: 1.1078x; 1.0612x over previous
"""Distributed Trainium2 (8 NeuronCores) Bass kernel for AdaptivePPOPolicyGNN.

Strategy (row-parallel dense GCN per the sharding hint):
 - Host prep: build the dense adjacency A (set semantics + self loops, values
   {0,1,2} -> exact in bf16), ship each core its row-block TRANSPOSED
   (AT_c = A[rows_c,:].T, [8192,1024] bf16) plus a replicated node-feature
   copy and tiny replicated/col-sharded MLP weights.
 - Device, per core: deg_c = row sums of A_c (PE ones-matmul while A streams
   into SBUF), AllGather deg -> dinv = rsqrt(deg); SpMM1/SpMM2 as
   feature-major TensorE matmuls out^T = xs^T @ A_c^T with xs stationary and
   A_c^T (resident in SBUF) as the moving operand; AllGather of the scaled
   hidden state between layers; exact streaming-softmax pooling combine via a
   130-float AllGather; actor logits col-sharded + AllGather; critic local.

Outputs: (action_probs [8192] f32, value scalar f32).
"""
import numpy as np
import ml_dtypes

from concourse import bass, bacc, mybir, tile, masks, bass_isa
from concourse.bass_utils import run_bass_kernel_spmd

BF16 = mybir.dt.bfloat16
FP8 = mybir.dt.float8e4
FP32 = mybir.dt.float32
AF = mybir.ActivationFunctionType
ALU = mybir.AluOpType

N, F, H, A_SZ = 8192, 64, 128, 8192
C, RPC = 8, 1024          # cores, rows per core
KT, MT = N // 128, RPC // 128  # 64 k-tiles, 8 m-tiles
LN_EPS = 1e-5
GROUPS = [list(range(C))]


def build_nc(stage=7):
    nc = bacc.Bacc(None, target_bir_lowering=False, num_devices=C)

    # ---- per-core external inputs -------------------------------------
    at = nc.declare_dram_parameter("at", [N, RPC], FP8, False)        # A[rows_c].T
    xsw = nc.declare_dram_parameter("xsw", [128, KT, F], BF16, False)  # x swizzled (p,t,f)
    xtc = nc.declare_dram_parameter("xtc", [F, RPC], BF16, False)      # x[rows_c].T
    g1w = nc.declare_dram_parameter("g1w", [F, H], BF16, False)
    g1b = nc.declare_dram_parameter("g1b", [H, 1], FP32, False)
    g2w = nc.declare_dram_parameter("g2w", [H, H], BF16, False)
    g2b = nc.declare_dram_parameter("g2b", [H, 1], FP32, False)
    resw = nc.declare_dram_parameter("resw", [F, H], BF16, False)
    resb = nc.declare_dram_parameter("resb", [H, 1], FP32, False)
    a1w = nc.declare_dram_parameter("a1w", [H, F], FP32, False)
    a1b = nc.declare_dram_parameter("a1b", [F, 1], FP32, False)
    a2wc = nc.declare_dram_parameter("a2wc", [F, RPC], BF16, False)    # a2_W col shard
    a2bc = nc.declare_dram_parameter("a2bc", [1, RPC], FP32, False)
    lng = nc.declare_dram_parameter("lng", [H, 1], FP32, False)
    lnb = nc.declare_dram_parameter("lnb", [H, 1], FP32, False)
    c1w = nc.declare_dram_parameter("c1w", [H, F], FP32, False)
    c1b = nc.declare_dram_parameter("c1b", [1, F], FP32, False)
    c2w = nc.declare_dram_parameter("c2w", [1, F], FP32, False)        # c2_W.T
    c2b = nc.declare_dram_parameter("c2b", [1, 1], FP32, False)
    out_probs = nc.declare_dram_parameter("probs", [C, RPC], FP32, True)
    out_value = nc.declare_dram_parameter("value", [1, 1], FP32, True)

    # ---- collective bounce buffers ------------------------------------
    deg_in = nc.dram_tensor("deg_in", [RPC], FP32)
    deg_out = nc.dram_tensor("deg_out", [N], FP32, addr_space="Shared")
    hs_in = nc.dram_tensor("hs_in", [128, MT, H], BF16)
    hs_out = nc.dram_tensor("hs_out", [C, 128, MT, H], BF16, addr_space="Shared")
    pool_in = nc.dram_tensor("pool_in", [130], FP32)
    pool_out = nc.dram_tensor("pool_out", [C, 130], FP32, addr_space="Shared")
    lg_in = nc.dram_tensor("lg_in", [RPC], FP32)
    lg_out = nc.dram_tensor("lg_out", [C, RPC], FP32, addr_space="Shared")

    def body(tc):
        with (
            tc.tile_pool(name="per", bufs=1) as per,      # persistent sbuf
            tc.tile_pool(name="wk", bufs=5) as wkp,       # recycled f32 [*,1024] tiles
            tc.tile_pool(name="psA", bufs=3, space="PSUM") as psA,   # 2-bank psums
            tc.tile_pool(name="psB", bufs=2, space="PSUM") as psB,   # 1-bank psums
        ):
            def wk(shape, dtype=FP32, name="wkt"):
                return wkp.tile(shape, dtype, tag="wk", name=name)

            # ---- phase 0 first: start streaming A immediately ----
            at_s = per.tile([128, KT, RPC], FP8, name="at_s")
            at_r = at[:, :].rearrange("(t p) m -> p t m", p=128)
            CH = 8  # k-tiles per DMA chunk, alternate the two HWDGE rings
            for i in range(KT // CH):
                eng = nc.sync if i % 2 == 0 else nc.scalar
                eng.dma_start(at_s[:, i * CH:(i + 1) * CH, :], at_r[:, i * CH:(i + 1) * CH, :])

            # constants
            ident = per.tile([128, 128], FP32, name="ident")
            masks.make_identity(nc, ident[:, :])
            ones_bf = per.tile([128, 1], BF16, name="ones_bf")
            nc.vector.memset(ones_bf[:, :], 1.0)
            ones_f8 = per.tile([128, 1], FP8, name="ones_f8")
            nc.vector.memset(ones_f8[:, :], 1.0)
            ones_f = per.tile([128, 1], FP32, name="ones_f")
            nc.vector.memset(ones_f[:, :], 1.0)
            ones8 = per.tile([8, 1], FP32, name="ones8")
            nc.vector.memset(ones8[:, :], 1.0)
            row1 = per.tile([1, 128], FP32, name="row1")
            nc.vector.memset(row1[:, :], 1.0)
            row1_bf = per.tile([1, 128], BF16, name="row1_bf")
            nc.vector.memset(row1_bf[:, :], 1.0)

            # small weight loads
            g1w_s = per.tile([F, H], BF16, name="g1w_s"); nc.gpsimd.dma_start(g1w_s[:, :], g1w[:, :])
            g2w_s = per.tile([H, H], BF16, name="g2w_s"); nc.gpsimd.dma_start(g2w_s[:, :], g2w[:, :])
            resw_s = per.tile([F, H], BF16, name="resw_s"); nc.gpsimd.dma_start(resw_s[:, :], resw[:, :])
            g1b_s = per.tile([H, 1], FP32, name="g1b_s"); nc.gpsimd.dma_start(g1b_s[:, :], g1b[:, :])
            g2b_s = per.tile([H, 1], FP32, name="g2b_s"); nc.gpsimd.dma_start(g2b_s[:, :], g2b[:, :])
            resb_s = per.tile([H, 1], FP32, name="resb_s"); nc.gpsimd.dma_start(resb_s[:, :], resb[:, :])
            a1w_s = per.tile([H, F], FP32, name="a1w_s"); nc.gpsimd.dma_start(a1w_s[:, :], a1w[:, :])
            a1b_s = per.tile([F, 1], FP32, name="a1b_s"); nc.gpsimd.dma_start(a1b_s[:, :], a1b[:, :])
            a2wc_s = per.tile([F, RPC], BF16, name="a2wc_s"); nc.gpsimd.dma_start(a2wc_s[:, :], a2wc[:, :])
            a2bc_s = per.tile([1, RPC], FP32, name="a2bc_s"); nc.gpsimd.dma_start(a2bc_s[:, :], a2bc[:, :])
            lng_s = per.tile([H, 1], FP32, name="lng_s"); nc.gpsimd.dma_start(lng_s[:, :], lng[:, :])
            lnb_s = per.tile([H, 1], FP32, name="lnb_s"); nc.gpsimd.dma_start(lnb_s[:, :], lnb[:, :])
            c1w_s = per.tile([H, F], FP32, name="c1w_s"); nc.gpsimd.dma_start(c1w_s[:, :], c1w[:, :])
            c1b_s = per.tile([1, F], FP32, name="c1b_s"); nc.gpsimd.dma_start(c1b_s[:, :], c1b[:, :])
            c2w_s = per.tile([1, F], FP32, name="c2w_s"); nc.gpsimd.dma_start(c2w_s[:, :], c2w[:, :])
            c2b_s = per.tile([1, 1], FP32, name="c2b_s"); nc.gpsimd.dma_start(c2b_s[:, :], c2b[:, :])

            xsw_s = per.tile([128, KT, F], BF16, name="xsw_s")
            nc.gpsimd.dma_start(xsw_s[:, :, :], xsw[:, :, :])
            xtc_s = per.tile([F, RPC], BF16, name="xtc_s")
            nc.gpsimd.dma_start(xtc_s[:, :], xtc[:, :])

            def finish_dbg(row_ap):
                # debug early-exit: write a [1, RPC] f32 row into probs[0]
                nc.gpsimd.dma_start(out_probs[0:1, :], row_ap)
            # deg: even k-tiles summed on PE (ones-matmul), odd tiles on DVE (bf16
            # adds are exact for these small-integer values), both folded into ps_deg.
            ps_deg = psA.tile([1, RPC], FP32, tag="mm", name="ps_deg")
            acc = per.tile([128, RPC], FP32, name="acc")
            for t in range(KT):
                if t % 2 == 0:
                    for h in range(2):
                        nc.tensor.matmul(ps_deg[:, h * 512:(h + 1) * 512], ones_f8[:, :],
                                         at_s[:, t, h * 512:(h + 1) * 512],
                                         start=(t == 0), stop=False)
                elif t == 1:
                    nc.vector.tensor_copy(acc[:, :], at_s[:, t, :])
                else:
                    nc.vector.tensor_add(acc[:, :], acc[:, :], at_s[:, t, :])
            for h in range(2):
                nc.tensor.matmul(ps_deg[:, h * 512:(h + 1) * 512], ones_f[:, :],
                                 acc[:, h * 512:(h + 1) * 512], start=False, stop=True)
            deg_row = wk([1, RPC], name="deg_row")
            nc.vector.tensor_copy(deg_row[:, :], ps_deg[:, :])
            if stage <= 1:
                finish_dbg(deg_row[:, :])
                return
            nc.gpsimd.dma_start(deg_in[:], deg_row[:, :])
            nc.gpsimd.collective_compute("AllGather", ALU.bypass, replica_groups=GROUPS,
                                         ins=[deg_in.ap().opt()], outs=[deg_out.ap().opt()])

            # ---- phase 1: dinv, D2, xs --------------------------------
            degT = per.tile([64, 128], FP32, name="degT")
            nc.gpsimd.dma_start(degT[:, :], deg_out[:].rearrange("(q j) -> q j", j=128))
            ps_t64 = psB.tile([128, 64], FP32, tag="tr", name="ps_t64")
            nc.tensor.transpose(ps_t64[:, :], degT[:, :], ident[0:64, 0:64])
            sq_t = per.tile([128, KT], FP32, name="sq_t")
            nc.scalar.activation(sq_t[:, :], ps_t64[:, :], AF.Ln)
            dinv_t = per.tile([128, KT], FP32, name="dinv_t")
            nc.scalar.activation(dinv_t[:, :], sq_t[:, :], AF.Exp, scale=-0.5)

            dsq_row = wk([1, RPC], name="dsq_row")
            nc.scalar.activation(dsq_row[:, :], deg_row[:, :], AF.Ln)
            dinv_row = wk([1, RPC], name="dinv_row")
            nc.scalar.activation(dinv_row[:, :], dsq_row[:, :], AF.Exp, scale=-0.5)
            ps_d2 = psA.tile([128, RPC], FP32, tag="mm", name="ps_d2")
            for h in range(2):
                nc.tensor.matmul(ps_d2[:, h * 512:(h + 1) * 512], row1[:, :],
                                 dinv_row[:, h * 512:(h + 1) * 512])
            d2_s = per.tile([128, RPC], FP32, name="d2_s")
            nc.vector.tensor_copy(d2_s[:, :], ps_d2[:, :])

            xs_s = per.tile([128, KT, F], BF16, name="xs_s")
            for t in range(KT):
                nc.vector.tensor_scalar_mul(xs_s[:, t, :], xsw_s[:, t, :], dinv_t[:, t:t + 1])

            if stage <= 2:
                drow = wk([1, RPC], name="drow")
                nc.vector.tensor_copy(drow[0:1, 0:KT], dinv_t[0:1, :])
                nc.vector.tensor_copy(drow[0:1, KT:2 * KT], d2_s[0:1, 0:KT])
                nc.vector.tensor_copy(drow[0:1, 128:1024], d2_s[0:1, 128:1024])
                finish_dbg(drow[:, :])
                return

            # ---- phase 2: SpMM1 + layer 1 -----------------------------
            ps_ax = psA.tile([F, RPC], FP32, tag="mm", name="ps_ax")
            for t in range(KT):
                for h in range(2):
                    nc.tensor.matmul(ps_ax[:, h * 512:(h + 1) * 512], xs_s[:, t, :],
                                     at_s[:, t, h * 512:(h + 1) * 512],
                                     start=(t == 0), stop=(t == KT - 1))
            axt = wk([F, RPC], BF16, name="axt")
            nc.vector.tensor_copy(axt[:, :], ps_ax[:, :])

            ps_res = psA.tile([H, RPC], FP32, tag="mm", name="ps_res")
            for h in range(2):
                nc.tensor.matmul(ps_res[:, h * 512:(h + 1) * 512], resw_s[:, :],
                                 xtc_s[:, h * 512:(h + 1) * 512])
            ps_g1 = psA.tile([H, RPC], FP32, tag="mm", name="ps_g1")
            for h in range(2):
                nc.tensor.matmul(ps_g1[:, h * 512:(h + 1) * 512], g1w_s[:, :],
                                 axt[:, h * 512:(h + 1) * 512])
            u1 = wk([H, RPC], name="u1")
            nc.vector.scalar_tensor_tensor(u1[:, :], ps_g1[:, :], 1.0, d2_s[:, :], ALU.mult, ALU.mult)
            r1 = wk([H, RPC], name="r1")
            nc.vector.tensor_scalar(r1[:, :], u1[:, :], g1b_s[:, 0:1], 0.0, ALU.add, ALU.max)
            h1t = wk([H, RPC], name="h1t")
            nc.vector.scalar_tensor_tensor(h1t[:, :], ps_res[:, :], resb_s[:, 0:1], r1[:, :], ALU.add, ALU.add)
            hst = wk([H, RPC], name="hst")
            nc.vector.tensor_mul(hst[:, :], h1t[:, :], d2_s[:, :])

            hs_nm = per.tile([128, MT, H], BF16, name="hs_nm")
            for mt in range(MT):
                ps_tr = psB.tile([128, 128], FP32, tag="tr", name="ps_tr")
                nc.tensor.transpose(ps_tr[:, :], hst[:, mt * 128:(mt + 1) * 128], ident[:, :])
                nc.vector.tensor_copy(hs_nm[:, mt, :], ps_tr[:, :])
            if stage <= 3:
                finish_dbg(hst[0:1, :])
                return
            nc.gpsimd.dma_start(hs_in[:, :, :], hs_nm[:, :, :])
            if stage == 41:
                rb_bf = per.tile([1, RPC], BF16, name="rb_bf")
                nc.gpsimd.dma_start(rb_bf[0:1, :], hs_in[0:1, :, :].rearrange("p t f -> p (t f)"))
                rb_f = wk([1, RPC], name="rb_f")
                nc.vector.tensor_copy(rb_f[:, :], rb_bf[:, :])
                finish_dbg(rb_f[:, :])
                return
            nc.gpsimd.collective_compute("AllGather", ALU.bypass, replica_groups=GROUPS,
                                         ins=[hs_in.ap().opt()], outs=[hs_out.ap().opt()])
            if stage == 42:
                rb_bf = per.tile([1, RPC], BF16, name="rb_bf")
                nc.gpsimd.dma_start(rb_bf[0:1, :], hs_out[2, 0:1, :, :].rearrange("p t f -> p (t f)"))
                rb_f = wk([1, RPC], name="rb_f")
                nc.vector.tensor_copy(rb_f[:, :], rb_bf[:, :])
                finish_dbg(rb_f[:, :])
                return
            hs_s = per.tile([128, C, MT, H], BF16, name="hs_s")
            for r in range(C):
                nc.gpsimd.dma_start(hs_s[:, r, :, :], hs_out[r, :, :, :])

            if stage <= 4:
                hrow = wk([1, RPC], name="hrow")
                nc.vector.tensor_copy(hrow[0:1, :], hs_s[0:1, 0, :, :].rearrange("p t f -> p (t f)"))
                finish_dbg(hrow[:, :])
                return

            # ---- phase 3: SpMM2 + layer 2 + pooling -------------------
            ps_o2 = psA.tile([H, RPC], FP32, tag="mm", name="ps_o2")
            for T in range(KT):
                for h in range(2):
                    nc.tensor.matmul(ps_o2[:, h * 512:(h + 1) * 512], hs_s[:, T // MT, T % MT, :],
                                     at_s[:, T, h * 512:(h + 1) * 512],
                                     start=(T == 0), stop=(T == KT - 1))
            o2 = wk([H, RPC], BF16, name="o2")
            nc.vector.tensor_copy(o2[:, :], ps_o2[:, :])
            ps_g2 = psA.tile([H, RPC], FP32, tag="mm", name="ps_g2")
            for h in range(2):
                nc.tensor.matmul(ps_g2[:, h * 512:(h + 1) * 512], g2w_s[:, :],
                                 o2[:, h * 512:(h + 1) * 512])
            u2 = wk([H, RPC], name="u2")
            nc.vector.scalar_tensor_tensor(u2[:, :], ps_g2[:, :], 1.0, d2_s[:, :], ALU.mult, ALU.mult)
            h2t = wk([H, RPC], BF16, name="h2t")
            nc.vector.tensor_scalar(h2t[:, :], u2[:, :], g2b_s[:, 0:1], 0.0, ALU.add, ALU.max)

            ps_s = psA.tile([1, RPC], FP32, tag="mm", name="ps_s")
            for h in range(2):
                nc.tensor.matmul(ps_s[:, h * 512:(h + 1) * 512], ones_bf[:, :],
                                 h2t[:, h * 512:(h + 1) * 512])
            m_c = per.tile([1, 1], FP32, name="m_c")
            nc.vector.reduce_max(m_c[:, :], ps_s[:, :], axis=mybir.AxisListType.X)
            neg_m = per.tile([1, 1], FP32, name="neg_m")
            nc.vector.tensor_scalar_mul(neg_m[:, :], m_c[:, :], -1.0)
            w_row = wk([1, RPC], BF16, name="w_row")
            den_c = per.tile([1, 1], FP32, name="den_c")
            nc.scalar.activation(w_row[:, :], ps_s[:, :], AF.Exp, bias=neg_m[:, 0:1])
            nc.vector.reduce_sum(den_c[:, :], w_row[:, :], axis=mybir.AxisListType.X)
            ps_wb = psA.tile([128, RPC], FP32, tag="mm", name="ps_wb")
            for h in range(2):
                nc.tensor.matmul(ps_wb[:, h * 512:(h + 1) * 512], row1_bf[:, :],
                                 w_row[:, h * 512:(h + 1) * 512])
            scr = wk([H, RPC], name="scr")
            num_c = per.tile([H, 1], FP32, name="num_c")
            nc.vector.tensor_mul(scr[:, :], h2t[:, :], ps_wb[:, :])
            nc.vector.reduce_sum(num_c[:, :], scr[:, :], axis=mybir.AxisListType.X)
            ps_trn = psB.tile([1, 128], FP32, tag="tr", name="ps_trn")
            nc.tensor.matmul(ps_trn[:, :], num_c[:, :], ident[:, :])
            pool_row = per.tile([1, 130], FP32, name="pool_row")
            nc.vector.tensor_copy(pool_row[:, 0:1], m_c[:, :])
            nc.vector.tensor_copy(pool_row[:, 1:2], den_c[:, :])
            nc.vector.tensor_copy(pool_row[:, 2:130], ps_trn[:, :])
            if stage <= 5:
                finish_dbg(h2t[0:1, :])
                return
            nc.gpsimd.dma_start(pool_in[:], pool_row[:, :])
            nc.gpsimd.collective_compute("AllGather", ALU.bypass, replica_groups=GROUPS,
                                         ins=[pool_in.ap().opt()], outs=[pool_out.ap().opt()])
            pool_s = per.tile([8, 130], FP32, name="pool_s")
            nc.gpsimd.dma_start(pool_s[:, :], pool_out[:, :])

            gmax8 = per.tile([8, 1], FP32, name="gmax8")
            nc.gpsimd.partition_all_reduce(gmax8[:, :], pool_s[:, 0:1], channels=8,
                                           reduce_op=bass_isa.ReduceOp.max)
            ngmax8 = per.tile([8, 1], FP32, name="ngmax8")
            nc.vector.tensor_scalar_mul(ngmax8[:, :], gmax8[:, :], -1.0)
            w8 = per.tile([8, 1], FP32, name="w8")
            nc.scalar.activation(w8[:, :], pool_s[:, 0:1], AF.Exp, bias=ngmax8[:, 0:1])
            scaled = per.tile([8, 129], FP32, name="scaled")
            nc.vector.tensor_scalar_mul(scaled[:, :], pool_s[:, 1:130], w8[:, 0:1])
            ps_cmb = psB.tile([128, 1], FP32, tag="tr", name="ps_cmb")
            nc.tensor.matmul(ps_cmb[:, :], scaled[:, 1:129], ones8[:, :])
            ps_den = psB.tile([1, 1], FP32, tag="tr", name="ps_den")
            nc.tensor.matmul(ps_den[:, :], scaled[:, 0:1], ones8[:, :])
            den_rec = per.tile([1, 1], FP32, name="den_rec")
            nc.vector.reciprocal(den_rec[:, :], ps_den[:, :])
            den_bc = per.tile([128, 1], FP32, name="den_bc")
            nc.gpsimd.partition_broadcast(den_bc[:, :], den_rec[:, :])
            g_pm = per.tile([128, 1], FP32, name="g_pm")
            nc.vector.tensor_scalar_mul(g_pm[:, :], ps_cmb[:, :], den_bc[:, 0:1])

            if stage <= 6:
                grow = wk([1, RPC], name="grow")
                nc.vector.memset(grow[:, :], 0.0)
                nc.vector.tensor_copy(grow[0:1, 0:8], pool_s[0:1, 0:8])
                finish_dbg(grow[:, :])
                # still run critic below (no more collectives)
            # ---- actor -----------------------------------------------
            run_actor = stage >= 7
            ps_z1 = psB.tile([F, 1], FP32, tag="tr", name="ps_z1")
            nc.tensor.matmul(ps_z1[:, :], a1w_s[:, :], g_pm[:, :])
            za = per.tile([F, 1], BF16, name="za")
            nc.vector.tensor_scalar(za[:, :], ps_z1[:, :], a1b_s[:, 0:1], 0.0, ALU.add, ALU.max)
            ps_lg = psA.tile([1, RPC], FP32, tag="mm", name="ps_lg")
            for h in range(2):
                nc.tensor.matmul(ps_lg[:, h * 512:(h + 1) * 512], za[:, :],
                                 a2wc_s[:, h * 512:(h + 1) * 512])
            lgr = wk([1, RPC], name="lgr")
            nc.vector.tensor_add(lgr[:, :], ps_lg[:, :], a2bc_s[:, :])
            if run_actor:
                nc.gpsimd.dma_start(lg_in[:], lgr[:, :])
                nc.gpsimd.collective_compute("AllGather", ALU.bypass, replica_groups=GROUPS,
                                             ins=[lg_in.ap().opt()], outs=[lg_out.ap().opt()])
            lg_s = wk([8, RPC], name="lg_s")
            if run_actor:
                nc.gpsimd.dma_start(lg_s[:, :], lg_out[:, :])
            else:
                nc.vector.memset(lg_s[:, :], 0.0)
            lmax8 = per.tile([8, 1], FP32, name="lmax8")
            nc.vector.reduce_max(lmax8[:, :], lg_s[:, :], axis=mybir.AxisListType.X)
            glm8 = per.tile([8, 1], FP32, name="glm8")
            nc.gpsimd.partition_all_reduce(glm8[:, :], lmax8[:, :], channels=8,
                                           reduce_op=bass_isa.ReduceOp.max)
            nglm8 = per.tile([8, 1], FP32, name="nglm8")
            nc.vector.tensor_scalar_mul(nglm8[:, :], glm8[:, :], -1.0)
            e8 = wk([8, RPC], name="e8")
            esum8 = per.tile([8, 1], FP32, name="esum8")
            nc.scalar.activation(e8[:, :], lg_s[:, :], AF.Exp, bias=nglm8[:, 0:1])
            nc.vector.reduce_sum(esum8[:, :], e8[:, :], axis=mybir.AxisListType.X)
            tot8 = per.tile([8, 1], FP32, name="tot8")
            nc.gpsimd.partition_all_reduce(tot8[:, :], esum8[:, :], channels=8,
                                           reduce_op=bass_isa.ReduceOp.add)
            rec8 = per.tile([8, 1], FP32, name="rec8")
            nc.vector.reciprocal(rec8[:, :], tot8[:, :])
            probs_s = wk([8, RPC], name="probs_s")
            nc.vector.tensor_scalar_mul(probs_s[:, :], e8[:, :], rec8[:, 0:1])
            if run_actor:
                nc.gpsimd.dma_start(out_probs[:, :], probs_s[:, :])

            # ---- critic ----------------------------------------------
            mu128 = per.tile([128, 1], FP32, name="mu128")
            nc.gpsimd.partition_all_reduce(mu128[:, :], g_pm[:, :], channels=128,
                                           reduce_op=bass_isa.ReduceOp.add)
            mu = per.tile([128, 1], FP32, name="mu")
            nc.vector.tensor_scalar_mul(mu[:, :], mu128[:, :], 1.0 / H)
            tdev = per.tile([128, 1], FP32, name="tdev")
            nc.vector.tensor_sub(tdev[:, :], g_pm[:, :], mu[:, :])
            sqd = per.tile([128, 1], FP32, name="sqd")
            nc.vector.tensor_mul(sqd[:, :], tdev[:, :], tdev[:, :])
            var128 = per.tile([128, 1], FP32, name="var128")
            nc.gpsimd.partition_all_reduce(var128[:, :], sqd[:, :], channels=128,
                                           reduce_op=bass_isa.ReduceOp.add)
            eps_t = per.tile([128, 1], FP32, name="eps_t")
            nc.vector.memset(eps_t[:, :], float(LN_EPS))
            sdev = per.tile([128, 1], FP32, name="sdev")
            nc.scalar.activation(sdev[:, :], var128[:, :], AF.Sqrt, bias=eps_t[:, 0:1], scale=float(1.0 / H))
            rsd = per.tile([128, 1], FP32, name="rsd")
            nc.vector.reciprocal(rsd[:, :], sdev[:, :])
            zn = per.tile([128, 1], FP32, name="zn")
            nc.vector.tensor_scalar(zn[:, :], tdev[:, :], rsd[:, 0:1], lng_s[:, 0:1], ALU.mult, ALU.mult)
            zn2 = per.tile([128, 1], FP32, name="zn2")
            nc.vector.tensor_scalar_add(zn2[:, :], zn[:, :], lnb_s[:, 0:1])
            ps_c1 = psB.tile([1, F], FP32, tag="tr", name="ps_c1")
            nc.tensor.matmul(ps_c1[:, :], zn2[:, :], c1w_s[:, :])
            cr = per.tile([1, F], FP32, name="cr")
            nc.vector.tensor_add(cr[:, :], ps_c1[:, :], c1b_s[:, :])
            cr2 = per.tile([1, F], FP32, name="cr2")
            nc.vector.tensor_relu(cr2[:, :], cr[:, :])
            scrv = per.tile([1, F], FP32, name="scrv")
            valp = per.tile([1, 1], FP32, name="valp")
            nc.vector.tensor_mul(scrv[:, :], cr2[:, :], c2w_s[:, :])
            nc.vector.reduce_sum(valp[:, :], scrv[:, :], axis=mybir.AxisListType.X)
            val2 = per.tile([1, 1], FP32, name="val2")
            nc.vector.tensor_scalar_add(val2[:, :], valp[:, :], c2b_s[:, 0:1])
            nc.gpsimd.dma_start(out_value[:, :], val2[:, :])

    with tile.TileContext(nc) as tc:
        body(tc)
    nc.compile()
    return nc


_NC_CACHE = {}


def _get_nc(stage=7):
    if stage not in _NC_CACHE:
        _NC_CACHE[stage] = build_nc(stage)
    return _NC_CACHE[stage]


def _prep_in_maps(node_features, edge_index):
    bf = ml_dtypes.bfloat16
    x = np.asarray(node_features, np.float32)
    ei = np.asarray(edge_index)

    f8 = mybir.dt.np(FP8)
    adj_bf = np.zeros((N, N), f8)
    adj_bf[ei[0], ei[1]] = 1.0
    idx = np.arange(N)
    adj_bf[idx, idx] = adj_bf[idx, idx].astype(np.float32) + 1.0

    x_bf = x.astype(bf)
    xsw_np = np.ascontiguousarray(x_bf.reshape(KT, 128, F).transpose(1, 0, 2))

    return adj_bf, x, xsw_np


def _run(inputs, trace=False, stage=7, **kwargs):
    nc = _get_nc(stage)
    f32 = lambda a: np.ascontiguousarray(np.asarray(a, np.float32))
    node_features = inputs["node_features"]
    edge_index = inputs["edge_index"]
    adj_bf, x, xsw_np = _prep_in_maps(node_features, edge_index)

    w = {k: f32(v) for k, v in inputs.items() if k not in ("node_features", "edge_index")}
    col = lambda a: f32(a).reshape(-1, 1)
    bfc = lambda a: np.ascontiguousarray(np.asarray(a, np.float32).astype(ml_dtypes.bfloat16))

    in_maps = []
    for c in range(C):
        r0, r1_ = c * RPC, (c + 1) * RPC
        m = {
            "at": np.ascontiguousarray(adj_bf[r0:r1_, :].T),
            "xsw": xsw_np,
            "xtc": bfc(x[r0:r1_, :].T),
            "g1w": bfc(w["g1_W"]), "g1b": col(w["g1_b"]),
            "g2w": bfc(w["g2_W"]), "g2b": col(w["g2_b"]),
            "resw": bfc(w["res_W"]), "resb": col(w["res_b"]),
            "a1w": w["a1_W"], "a1b": col(w["a1_b"]),
            "a2wc": bfc(w["a2_W"][:, r0:r1_]),
            "a2bc": f32(w["a2_b"][r0:r1_]).reshape(1, RPC),
            "lng": col(w["ln_g"]), "lnb": col(w["ln_b"]),
            "c1w": w["c1_W"], "c1b": f32(w["c1_b"]).reshape(1, F),
            "c2w": np.ascontiguousarray(w["c2_W"].T), "c2b": f32(w["c2_b"]).reshape(1, 1),
        }
        in_maps.append(m)

    res = run_bass_kernel_spmd(nc, in_maps, core_ids=list(range(C)), trace=trace, **kwargs)
    return res


def kernel(**inputs):
    res = _run(inputs, trace=False)
    out = res.results[0]
    probs = np.asarray(out["probs"], np.float32).reshape(A_SZ)
    value = np.float32(np.asarray(out["value"]).reshape(-1)[0])
    return probs, value
